# revision 7
# baseline (speedup 1.0000x reference)
"""Trainium2 Bass kernel for nn_PredictingModule (gnn_message_passing).

Strategy: shard edges (data parallel) across 8 cores; node embedding
tables replicated per core (converted to bf16 on host). Per 512-edge
batch on each core:
  - 2 indirect-DMA gathers (256B bf16 rows) for src/dst node vectors
  - PE transposes gathered tiles to feature-major layout
  - bf16 matmuls (fp32 PSUM accum) for the 256->128->32->1 MLP
  - ACT relu+bias / sigmoid, per-batch output store
"""

import numpy as np
import ml_dtypes

D = 128
BATCH = 512
SUB = BATCH // 128  # 4 sub-tiles of 128 edges
E = 500_000
N_CORES = 8
NE = E // N_CORES  # 62500 edges per core per etype
NB = -(-NE // BATCH)  # 123 batches
NEP = NB * BATCH  # 62976 padded
N_USER = 1_000_000
N_ITEM = 500_000

bf16 = ml_dtypes.bfloat16


def build_nc(nb=NB, n_user=N_USER, n_item=N_ITEM, group=2):
    import concourse.bass as bass
    import concourse.mybir as mybir
    import concourse.tile as tile
    from concourse import bacc
    from concourse.masks import make_identity

    dt = mybir.dt
    nc = bacc.Bacc("TRN2", target_bir_lowering=False, debug=False)

    hu = nc.declare_dram_parameter("hu", [n_user, D], dt.bfloat16, isOutput=False)
    hi = nc.declare_dram_parameter("hi", [n_item, D], dt.bfloat16, isOutput=False)
    # idx layout: [128, nb*SUB] int32; column b*SUB+c holds lane indices for
    # edges b*BATCH + c*128 + p
    idxs = {}
    for name in ("ia0", "ib0", "ia1", "ib1"):
        idxs[name] = nc.declare_dram_parameter(
            name, [128, nb * SUB], dt.int32, isOutput=False
        )
    wa = nc.declare_dram_parameter("wa", [D, D], dt.bfloat16, isOutput=False)
    wb = nc.declare_dram_parameter("wb", [D, D], dt.bfloat16, isOutput=False)
    w2 = nc.declare_dram_parameter("w2", [D, 32], dt.bfloat16, isOutput=False)
    w3 = nc.declare_dram_parameter("w3", [32, 1], dt.bfloat16, isOutput=False)
    b1 = nc.declare_dram_parameter("b1", [D, 1], dt.float32, isOutput=False)
    b2 = nc.declare_dram_parameter("b2", [32, 1], dt.float32, isOutput=False)
    b3 = nc.declare_dram_parameter("b3", [1, 1], dt.float32, isOutput=False)
    out0 = nc.declare_dram_parameter("out0", [nb, BATCH], dt.float32, isOutput=True)
    out1 = nc.declare_dram_parameter("out1", [nb, BATCH], dt.float32, isOutput=True)

    with tile.TileContext(nc) as tc:
        with (
            tc.tile_pool(name="const", bufs=1) as cpool,
            tc.tile_pool(name="g", bufs=6) as gpool,
            tc.tile_pool(name="xs", bufs=4) as xpool,
            tc.tile_pool(name="ys", bufs=4) as ypool,
            tc.tile_pool(name="os", bufs=4) as opool,
            tc.tile_pool(name="pxt", bufs=2, space="PSUM") as pxt,
            tc.tile_pool(name="py1", bufs=2, space="PSUM") as py1,
            tc.tile_pool(name="py2", bufs=2, space="PSUM") as py2,
            tc.tile_pool(name="py3", bufs=2, space="PSUM") as py3,
        ):
            wa_t = cpool.tile([D, D], dt.bfloat16)
            nc.sync.dma_start(out=wa_t[:], in_=wa[:])
            wb_t = cpool.tile([D, D], dt.bfloat16)
            nc.sync.dma_start(out=wb_t[:], in_=wb[:])
            w2_t = cpool.tile([D, 32], dt.bfloat16)
            nc.sync.dma_start(out=w2_t[:], in_=w2[:])
            w3_t = cpool.tile([32, 1], dt.bfloat16)
            nc.sync.dma_start(out=w3_t[:], in_=w3[:])
            b1_t = cpool.tile([D, 1], dt.float32)
            nc.sync.dma_start(out=b1_t[:], in_=b1[:])
            b2_t = cpool.tile([32, 1], dt.float32)
            nc.sync.dma_start(out=b2_t[:], in_=b2[:])
            b3_t = cpool.tile([1, 1], dt.float32)
            nc.sync.dma_start(out=b3_t[:], in_=b3[:])
            ident = cpool.tile([128, 128], dt.bfloat16)
            make_identity(nc, ident[:])

            idx_t = {}
            for name in idxs:
                t = cpool.tile([128, nb * SUB], dt.int32, tag=f"idx_{name}")
                nc.sync.dma_start(out=t[:], in_=idxs[name][:])
                idx_t[name] = t

            for et in range(2):
                tabA, tabB = (hu, hi) if et == 0 else (hi, hu)
                ia = idx_t["ia0"] if et == 0 else idx_t["ia1"]
                ib = idx_t["ib0"] if et == 0 else idx_t["ib1"]
                out_d = out0 if et == 0 else out1

                for b in range(nb):
                    gA = gpool.tile([128, SUB, D], dt.bfloat16, tag="gA")
                    gB = gpool.tile([128, SUB, D], dt.bfloat16, tag="gB")
                    for c in range(SUB):
                        nc.gpsimd.indirect_dma_start(
                            out=gA[:, c, :],
                            out_offset=None,
                            in_=tabA[:],
                            in_offset=bass.IndirectOffsetOnAxis(
                                ap=ia[:, b * SUB + c : b * SUB + c + 1], axis=0
                            ),
                        )
                        nc.gpsimd.indirect_dma_start(
                            out=gB[:, c, :],
                            out_offset=None,
                            in_=tabB[:],
                            in_offset=bass.IndirectOffsetOnAxis(
                                ap=ib[:, b * SUB + c : b * SUB + c + 1], axis=0
                            ),
                        )

                    # transpose both gathered tiles into one PSUM bank
                    xt = pxt.tile([128, 2 * BATCH], dt.bfloat16, tag="xt")
                    for c in range(SUB):
                        nc.tensor.transpose(
                            out=xt[:, c * 128 : (c + 1) * 128],
                            in_=gA[:, c, :],
                            identity=ident[:],
                        )
                    for c in range(SUB):
                        nc.tensor.transpose(
                            out=xt[:, BATCH + c * 128 : BATCH + (c + 1) * 128],
                            in_=gB[:, c, :],
                            identity=ident[:],
                        )

                    xa_s = xpool.tile([128, BATCH], dt.bfloat16, tag="xa")
                    nc.vector.tensor_copy(out=xa_s[:], in_=xt[:, :BATCH])
                    xb_s = xpool.tile([128, BATCH], dt.bfloat16, tag="xb")
                    nc.vector.tensor_copy(out=xb_s[:], in_=xt[:, BATCH:])

                    y1p = py1.tile([128, BATCH], dt.float32, tag="y1")
                    nc.tensor.matmul(
                        out=y1p[:], lhsT=wa_t[:], rhs=xa_s[:], start=True, stop=False
                    )
                    nc.tensor.matmul(
                        out=y1p[:], lhsT=wb_t[:], rhs=xb_s[:], start=False, stop=True
                    )
                    y1s = ypool.tile([128, BATCH], dt.bfloat16, tag="y1s")
                    nc.scalar.activation(
                        out=y1s[:],
                        in_=y1p[:],
                        func=mybir.ActivationFunctionType.Relu,
                        bias=b1_t[:],
                    )

                    y2p = py2.tile([32, BATCH], dt.float32, tag="y2")
                    nc.tensor.matmul(out=y2p[:], lhsT=w2_t[:], rhs=y1s[:])
                    y2s = ypool.tile([32, BATCH], dt.bfloat16, tag="y2s")
                    nc.scalar.activation(
                        out=y2s[:],
                        in_=y2p[:],
                        func=mybir.ActivationFunctionType.Relu,
                        bias=b2_t[:],
                    )

                    y3p = py3.tile([1, BATCH], dt.float32, tag="y3")
                    nc.tensor.matmul(out=y3p[:], lhsT=w3_t[:], rhs=y2s[:])
                    sig = opool.tile([1, BATCH], dt.float32, tag="sig")
                    nc.scalar.activation(
                        out=sig[:],
                        in_=y3p[:],
                        func=mybir.ActivationFunctionType.Sigmoid,
                        bias=b3_t[:],
                    )
                    nc.sync.dma_start(out=out_d[b, :], in_=sig[:])
    nc.compile()
    return nc


def _prep_idx(idx, ne, nb):
    """[ne] int -> [128, nb*SUB] int32 in kernel layout (pad with 0)."""
    pad = np.zeros(nb * BATCH, dtype=np.int32)
    pad[:ne] = np.asarray(idx, dtype=np.int32)[:ne]
    # edge e = b*BATCH + c*128 + p  ->  [p, b*SUB + c]
    return np.ascontiguousarray(
        pad.reshape(nb, SUB, 128).transpose(2, 0, 1).reshape(128, nb * SUB)
    )


def build_in_maps(h_user, h_item, src_ui, dst_ui, src_iu, dst_iu, W1, b1, W2, b2, W3, b3):
    h_user = np.asarray(h_user, dtype=np.float32)
    h_item = np.asarray(h_item, dtype=np.float32)
    src_ui = np.asarray(src_ui).astype(np.int32)
    dst_ui = np.asarray(dst_ui).astype(np.int32)
    src_iu = np.asarray(src_iu).astype(np.int32)
    dst_iu = np.asarray(dst_iu).astype(np.int32)
    W1 = np.asarray(W1, dtype=np.float32)
    b1 = np.asarray(b1, dtype=np.float32)
    W2 = np.asarray(W2, dtype=np.float32)
    b2 = np.asarray(b2, dtype=np.float32)
    W3 = np.asarray(W3, dtype=np.float32)
    b3 = np.asarray(b3, dtype=np.float32)

    hu_bf = h_user.astype(bf16)
    hi_bf = h_item.astype(bf16)
    wa_bf = np.ascontiguousarray(W1[:D]).astype(bf16)
    wb_bf = np.ascontiguousarray(W1[D:]).astype(bf16)
    w2_bf = W2.astype(bf16)
    w3_bf = W3.astype(bf16)
    b1c = np.ascontiguousarray(b1.reshape(D, 1))
    b2c = np.ascontiguousarray(b2.reshape(32, 1))
    b3c = np.ascontiguousarray(b3.reshape(1, 1))

    in_maps = []
    for c in range(N_CORES):
        sl = slice(c * NE, (c + 1) * NE)
        in_maps.append(
            {
                "hu": hu_bf,
                "hi": hi_bf,
                "ia0": _prep_idx(src_ui[sl], NE, NB),
                "ib0": _prep_idx(dst_ui[sl], NE, NB),
                "ia1": _prep_idx(src_iu[sl], NE, NB),
                "ib1": _prep_idx(dst_iu[sl], NE, NB),
                "wa": wa_bf,
                "wb": wb_bf,
                "w2": w2_bf,
                "w3": w3_bf,
                "b1": b1c,
                "b2": b2c,
                "b3": b3c,
            }
        )
    return in_maps


def kernel(h_user, h_item, src_ui, dst_ui, src_iu, dst_iu, W1, b1, W2, b2, W3, b3):
    from concourse.bass_utils import run_bass_kernel_spmd

    in_maps = build_in_maps(
        h_user, h_item, src_ui, dst_ui, src_iu, dst_iu, W1, b1, W2, b2, W3, b3
    )
    nc = build_nc()
    out = run_bass_kernel_spmd(nc, in_maps, list(range(N_CORES)))
    global _last_results
    _last_results = out
    res = out.results

    r_ui = np.concatenate(
        [res[c]["out0"].reshape(-1)[:NE] for c in range(N_CORES)]
    ).reshape(E, 1)
    r_iu = np.concatenate(
        [res[c]["out1"].reshape(-1)[:NE] for c in range(N_CORES)]
    ).reshape(E, 1)
    return (r_ui.astype(np.float32), r_iu.astype(np.float32))
